# revision 1
# baseline (speedup 1.0000x reference)
"""Trainium2 Bass kernel for CausalSelfAttention with LoRA (B=4, S=2048,
D=1024, H=16, Dh=64, rank=16), sharded over 8 NeuronCores.

Sharding: batch (4-way) x head-group (2-way). Core c handles batch c//2 and
heads (c%2)*8 .. (c%2)*8+7 (512 of the 1024 channels). Each core computes its
partial output projection; the host sums the two partials per batch element.

Host-side prep (free w.r.t. device time):
  - LoRA folded into the weights: W_eff = W + (1/rank) * b @ a  (fp64).
  - Weights/activations pre-transposed + cast to bf16 in the exact SBUF
    layouts the kernel wants.
  - 1/sqrt(Dh) folded into the Q projection weights.

Device algorithm (per core), all matmuls bf16 with fp32 PSUM accumulate:
  QT = WqT.T @ xT   [512ch, 2048tok] (transposed layout, ch on partitions)
  KT likewise; V = xT.T @ WvT [2048tok, 512ch] (token-major).
  Per head-pair, per 512-wide q block, loop over 128-wide k tiles (causal
  lower-triangle only):
    scoresT[k, q] = KT_h.T @ QT_h     (two heads row-packed in the PE array)
    attnT = exp(scoresT)  on ScalarE (scores bounded ~|4|, no max needed)
    diagonal tiles: multiply by triangular 0/1 mask on VectorE
    ctxT += V_h.T @ attnT             (4x column-packed in the PE array)
    den  += ones.T @ attnT            (softmax denominator, 2x col-packed)
  normalize: ctxT *= broadcast(1/den) (recip on VectorE, broadcast via PE)
  out_partial = ctxT.T @ WoT          (q-major, fp32, DMA'd to HBM)
"""

import os
import sys

sys.path.insert(0, "/opt/trn_rl_repo")

import numpy as np
import ml_dtypes

bf16np = ml_dtypes.bfloat16

D, H, Dh, R = 1024, 16, 64, 16
S, B = 2048, 4
SCALING = 1.0 / R
N_CORES = 8

_compiled = {}


def _build_nc():
    import concourse.bass as bass
    import concourse.tile as tile
    from concourse import mybir

    fp32 = mybir.dt.float32
    bf16 = mybir.dt.bfloat16

    nc = bass.Bass()

    xt_d = nc.dram_tensor("xt", [128, 8, S], bf16, kind="ExternalInput")
    wqt_d = nc.dram_tensor("wqt", [128, 8, 512], bf16, kind="ExternalInput")
    wkt_d = nc.dram_tensor("wkt", [128, 8, 512], bf16, kind="ExternalInput")
    wvt_d = nc.dram_tensor("wvt", [128, 8, 512], bf16, kind="ExternalInput")
    wot_d = nc.dram_tensor("wot", [128, 4, D], bf16, kind="ExternalInput")
    tri_d = nc.dram_tensor("tri", [128, 128], bf16, kind="ExternalInput")
    out_d = nc.dram_tensor("out", [16, 128, D], mybir.dt.float32, kind="ExternalOutput")

    with tile.TileContext(nc) as tc:
        with (
            tc.tile_pool(name="consts", bufs=1) as consts,
            tc.tile_pool(name="acts", bufs=1) as acts,
            tc.tile_pool(name="attn", bufs=4) as attn_pool,
            tc.tile_pool(name="small", bufs=2) as small,
            tc.tile_pool(name="ostage", bufs=3) as ostage,
            tc.tile_pool(name="ps_sc", bufs=2, space="PSUM") as ps_sc,
            tc.tile_pool(name="ps_proj", bufs=1, space="PSUM") as ps_proj,
            tc.tile_pool(name="ps_ctx", bufs=2, space="PSUM") as ps_ctx,
            tc.tile_pool(name="ps_aux", bufs=1, space="PSUM") as ps_aux,
            tc.tile_pool(name="dram", bufs=2, space="DRAM") as dram,
        ):
            # ---- load constants (wvt + x first so V-proj can start early) ----
            wvt = consts.tile([128, 8, 512], bf16, tag="wvt")
            nc.sync.dma_start(out=wvt, in_=wvt_d[:])
            xt = consts.tile([128, 8, S], bf16, tag="xt")
            for k in range(8):
                nc.sync.dma_start(out=xt[:, k, :], in_=xt_d[:, k, :])
            wqt = consts.tile([128, 8, 512], bf16, tag="wqt")
            nc.sync.dma_start(out=wqt, in_=wqt_d[:])
            wkt = consts.tile([128, 8, 512], bf16, tag="wkt")
            nc.sync.dma_start(out=wkt, in_=wkt_d[:])
            wot = consts.tile([128, 4, D], bf16, tag="wot")
            nc.sync.dma_start(out=wot, in_=wot_d[:])
            tri = consts.tile([128, 128], bf16, tag="tri")
            nc.sync.dma_start(out=tri, in_=tri_d[:])
            ones = consts.tile([128, 64], bf16, tag="ones")
            nc.vector.memset(ones, 1.0)
            warm = consts.tile([128, 512], bf16, tag="warm")
            nc.vector.memset(warm, 0.5)

            # ---- PE warm-up: junk matmuls while DMAs land, so the HAM clock
            # gate reaches 8/8 before real work (and PE never idles >3us) ----
            warm_ps = ps_aux.tile([128, 512], fp32, tag="aux", name="warm_ps")
            for _ in range(36):
                nc.tensor.matmul(
                    warm_ps[0:64, :],
                    warm[:, 0:64],
                    warm,
                    start=True,
                    stop=True,
                    skip_group_check=True,
                )

            qt = acts.tile([128, 4, S], bf16, tag="qt")
            ktt = acts.tile([128, 4, S], bf16, tag="ktt")
            v = acts.tile([128, 16, 512], bf16, tag="v")
            ctxt = acts.tile([128, 4, S], bf16, tag="ctxt")

            def v_proj(tt):
                # V projection for one token tile (all channel groups at once)
                vps_t = ps_sc.tile([128, 2, 512], fp32, tag="sc", name="vps")
                ps = vps_t[:, 0, :]
                for k in range(8):
                    nc.tensor.matmul(
                        ps,
                        xt[:, k, tt * 128:(tt + 1) * 128],
                        wvt[:, k, :],
                        start=(k == 0),
                        stop=(k == 7),
                    )
                nc.vector.tensor_copy(v[:, tt, :], ps)

            def qk_proj(p):
                for tb in range(4):
                    ps = ps_proj.tile([128, 512], fp32, tag="proj", name="qk_ps")
                    for k in range(8):
                        nc.tensor.matmul(
                            ps,
                            wqt[:, k, p * 128:(p + 1) * 128],
                            xt[:, k, tb * 512:(tb + 1) * 512],
                            start=(k == 0),
                            stop=(k == 7),
                        )
                    nc.vector.tensor_copy(qt[:, p, tb * 512:(tb + 1) * 512], ps)
                    ps = ps_proj.tile([128, 512], fp32, tag="proj", name="qk_ps")
                    for k in range(8):
                        nc.tensor.matmul(
                            ps,
                            wkt[:, k, p * 128:(p + 1) * 128],
                            xt[:, k, tb * 512:(tb + 1) * 512],
                            start=(k == 0),
                            stop=(k == 7),
                        )
                    nc.vector.tensor_copy(ktt[:, p, tb * 512:(tb + 1) * 512], ps)

            def oproj_inline(qt_i):
                # one output-projection token tile through the single proj
                # slot; fills PE gaps in the surrounding attention stream
                for db in range(2):
                    ps = ps_proj.tile([128, 512], fp32, tag="proj", name="op_ps")
                    for gg in range(4):
                        nc.tensor.matmul(
                            ps,
                            ctxt[:, gg, qt_i * 128:(qt_i + 1) * 128],
                            wot[:, gg, db * 512:(db + 1) * 512],
                            start=(gg == 0),
                            stop=(gg == 3),
                        )
                    st = ostage.tile([128, 512], fp32, tag="ostage")
                    nc.vector.tensor_copy(st, ps)
                    nc.sync.dma_start(
                        out=out_d[qt_i, :, db * 512:(db + 1) * 512], in_=st
                    )

            def attention(p, qb):
                    ctx_ps = ps_ctx.tile([128, 512], fp32, tag="ctx")
                    aux = ps_aux.tile([128, 512], fp32, tag="aux")
                    kt_hi = 4 * (qb + 1)
                    for kt in range(kt_hi):
                        j = kt - 4 * qb
                        c0 = 128 * j if j >= 0 else 0
                        sc = ps_sc.tile([128, 2, 512], fp32, tag="sc")
                        for s in range(2):
                            hp = slice(s * 64, (s + 1) * 64)
                            nc.tensor.matmul(
                                sc[:, s, c0:],
                                ktt[hp, p, kt * 128:(kt + 1) * 128],
                                qt[hp, p, qb * 512 + c0:(qb + 1) * 512],
                                start=True,
                                stop=True,
                                tile_position=(s * 64, 0),
                            )
                        at = attn_pool.tile([128, 2, 512], bf16, tag="at")
                        nc.scalar.activation(
                            out=at[:, :, c0:],
                            in_=sc[:, :, c0:],
                            func=mybir.ActivationFunctionType.Exp,
                        )
                        if j >= 0:
                            tri_b = bass.AP(
                                tensor=tri.tensor,
                                offset=tri.offset,
                                ap=[tri.ap[0], [0, 2], tri.ap[1]],
                            )
                            nc.vector.tensor_mul(
                                at[:, :, c0:c0 + 128], at[:, :, c0:c0 + 128], tri_b
                            )
                        first = kt == 0
                        last = kt == kt_hi - 1
                        for s in range(2):
                            for hh in range(2):
                                co = p * 128 + s * 64 + hh * 32
                                nc.tensor.matmul(
                                    ctx_ps[s * 64 + hh * 32:s * 64 + (hh + 1) * 32, c0:],
                                    v[:, kt, co:co + 32],
                                    at[:, s, c0:],
                                    start=first,
                                    stop=last,
                                    tile_position=(0, s * 64 + hh * 32),
                                )
                            nc.tensor.matmul(
                                aux[32 * s:32 * s + 1, c0:],
                                ones[:, 0:1],
                                at[:, s, c0:],
                                start=first,
                                stop=last,
                                skip_group_check=True,
                                tile_position=(0, 32 * s),
                            )
                    # normalization for (p, qb): 1/den = exp(-ln(den)) on the
                    # scalar engine (both funcs live in one ACT table set; DVE
                    # reciprocal is 3.3us and blocks the queue), then broadcast
                    # each head's row across its 64 partitions via a DRAM
                    # bounce (DMA supports partition-broadcast from DRAM)
                    ld = small.tile([33, 512], fp32, tag="ld")
                    nc.scalar.activation(
                        out=ld, in_=aux[0:33, :], func=mybir.ActivationFunctionType.Ln
                    )
                    rec = small.tile([33, 512], fp32, tag="rec")
                    nc.scalar.activation(
                        out=rec, in_=ld,
                        func=mybir.ActivationFunctionType.Exp, scale=-1.0,
                    )
                    dscr = dram.tile([2, 512], fp32, tag="dscr")
                    nc.sync.dma_start(out=dscr[0:1, :], in_=rec[0:1, :])
                    nc.sync.dma_start(out=dscr[1:2, :], in_=rec[32:33, :])
                    bc_sb = small.tile([128, 512], fp32, tag="bcsb")
                    nc.sync.dma_start(
                        out=bc_sb[0:64], in_=dscr[0:1, :].to_broadcast((64, 512))
                    )
                    nc.sync.dma_start(
                        out=bc_sb[64:128], in_=dscr[1:2, :].to_broadcast((64, 512))
                    )
                    nc.vector.tensor_mul(
                        ctxt[:, p, qb * 512:(qb + 1) * 512], ctx_ps, bc_sb
                    )

            # ---- schedule: pair 0 with just-in-time V projection (earliest
            # possible exp start), pair 1 qb-major, then pairs 2+3 interleaved
            # per q block with the output projection of each finished q block
            # emitted inline as PE gap-filler. ----
            qk_proj(0)
            for qb in range(4):
                for tt in range(4 * qb, 4 * qb + 4):
                    v_proj(tt)
                attention(0, qb)
            qk_proj(1)
            for qb in range(4):
                attention(1, qb)
            qk_proj(2)
            qk_proj(3)
            for qb in range(4):
                attention(2, qb)
                attention(3, qb)
                if qb < 3:
                    for qt_i in range(4 * qb, 4 * qb + 4):
                        oproj_inline(qt_i)

            # dense tail for the last q block: two accumulator chains per sc
            # tile (sc slots are free once attention is done)
            for qt_i in range(12, 16):
                ops_t = ps_sc.tile([128, 2, 512], fp32, tag="sc", name="ops")
                for db in range(2):
                    ps = ops_t[:, db, :]
                    for gg in range(4):
                        nc.tensor.matmul(
                            ps,
                            ctxt[:, gg, qt_i * 128:(qt_i + 1) * 128],
                            wot[:, gg, db * 512:(db + 1) * 512],
                            start=(gg == 0),
                            stop=(gg == 3),
                        )
                st = ostage.tile([128, 2, 512], fp32, tag="ostage2")
                nc.vector.tensor_copy(st, ops_t)
                nc.sync.dma_start(out=out_d[qt_i, :, :], in_=st.rearrange("p a b -> p (a b)"))

    _fix_matmul_waits(nc, mybir)
    return nc


_WAIT_LIMITS = {"InstISA": 0}


def _fix_matmul_waits(nc, mybir):
    """Walrus encodes at most one sync-wait command on compute-engine datapath
    instructions (MM/TT/ACT/...). Split excess waits into standalone
    InstEventSemaphore waits on the same engine immediately before the
    instruction — semantically identical (same engine stream, same point)."""
    import bass_rust

    counter = [0]

    def make_wait(engine, w):
        counter[0] += 1
        ev = mybir.InstEventSemaphore(name=f"W-split-{counter[0]}", ins=[], outs=[])
        ev.engine = engine
        ev.sync_info = bass_rust.SyncInfo(on_wait=[w], on_update=[])
        return ev

    for blk in nc.m.functions[0].blocks:
        insts = list(blk.instructions)
        out = []
        changed = False
        for ins in insts:
            si = ins.sync_info
            limit = _WAIT_LIMITS.get(type(ins).__name__, 1)
            if si is not None and len(si.on_wait) > limit:
                waits = list(si.on_wait)
                extra, keep = waits[:-limit], waits[-limit:]
                for w in extra:
                    out.append(make_wait(ins.engine, w))
                si.on_wait = keep
                ins.sync_info = si
                changed = True
            out.append(ins)
        if changed:
            blk.instructions = out


def _get_nc():
    if "nc" not in _compiled:
        _compiled["nc"] = _build_nc()
    return _compiled["nc"]


def _fold(w, a, b):
    return w.astype(np.float64) + SCALING * (
        b.astype(np.float64) @ a.astype(np.float64)
    )


def _prep_in_maps(inputs):
    x = np.asarray(inputs["x"], np.float32)
    wq_e = _fold(inputs["wq"], inputs["aq"], inputs["bq"])
    wk_e = _fold(inputs["wk"], inputs["ak"], inputs["bk"])
    wv_e = _fold(inputs["wv"], inputs["av"], inputs["bv"])
    wo_e = _fold(inputs["wo"], inputs["ao"], inputs["bo"])

    tri = np.triu(np.ones((128, 128), np.float32)).astype(bf16np)

    in_maps = []
    for c in range(N_CORES):
        b, g = c // 2, c % 2
        gs = slice(g * 512, (g + 1) * 512)
        xt = (
            x[b].T.reshape(8, 128, S).transpose(1, 0, 2).astype(bf16np)
        )
        wqt = (
            (wq_e[gs].T * 0.125).reshape(8, 128, 512).transpose(1, 0, 2).astype(bf16np)
        )
        wkt = wk_e[gs].T.reshape(8, 128, 512).transpose(1, 0, 2).astype(bf16np)
        wvt = wv_e[gs].T.reshape(8, 128, 512).transpose(1, 0, 2).astype(bf16np)
        wot = wo_e[:, gs].T.reshape(4, 128, D).transpose(1, 0, 2).astype(bf16np)
        in_maps.append(
            dict(
                xt=np.ascontiguousarray(xt),
                wqt=np.ascontiguousarray(wqt),
                wkt=np.ascontiguousarray(wkt),
                wvt=np.ascontiguousarray(wvt),
                wot=np.ascontiguousarray(wot),
                tri=tri,
            )
        )
    return in_maps


def run(inputs, trace=False, **kw):
    """Run on 8 cores; returns (full_output, BassKernelResults)."""
    from concourse.bass_utils import run_bass_kernel_spmd

    nc = _get_nc()
    in_maps = _prep_in_maps(inputs)
    res = run_bass_kernel_spmd(
        nc, in_maps, core_ids=list(range(N_CORES)), trace=trace, **kw
    )
    full = np.zeros((B, S, D), np.float32)
    for b in range(B):
        o0 = np.asarray(res.results[2 * b]["out"], np.float32).reshape(S, D)
        o1 = np.asarray(res.results[2 * b + 1]["out"], np.float32).reshape(S, D)
        full[b] = o0 + o1
    return full, res


def kernel(**inputs):
    full, _ = run(inputs, trace=False)
    return full



# revision 12
# speedup vs baseline: 1.0310x; 1.0310x over previous
"""Trainium2 Bass kernel for CausalSelfAttention with LoRA (B=4, S=2048,
D=1024, H=16, Dh=64, rank=16), sharded over 8 NeuronCores.

Sharding: batch (4-way) x head-group (2-way). Core c handles batch c//2 and
heads (c%2)*8 .. (c%2)*8+7 (512 of the 1024 channels). Each core computes its
partial output projection; the host sums the two partials per batch element.

Host-side prep (free w.r.t. device time):
  - LoRA folded into the weights: W_eff = W + (1/rank) * b @ a  (fp64).
  - Weights/activations pre-transposed + cast to bf16 in the exact SBUF
    layouts the kernel wants; 1/sqrt(Dh) folded into the Q weights.
  - x stored token-chunk-major so compute can start before all of x lands.

Device algorithm (per core), all matmuls bf16 with fp32 PSUM accumulate:
  QT = WqT.T @ xT   [512ch, 2048tok] (ch on partitions)
  KT likewise; V = xT.T @ WvT [2048tok, 512ch] (token-major).
  Attention runs as 16 (head-pair p, 512-q-block qb) "segments", two at a
  time in a wavefront order; per 128-k tile:
    scoresT[k, q] = KT_h.T @ QT_h  (2 heads row-packed in the PE)
    attnT = exp(scoresT) on ScalarE (scores bounded, no max pass)
    diagonal tiles: multiply by triangular 0/1 mask on VectorE
    ctxT += V_h.T @ attnT          (4x column-packed in the PE)
    acc  += attnT                  (softmax denominator partials on VectorE)
  Per segment end: den = ones.T @ acc (one thin matmul pair), 1/den via a
  single custom-DVE approximate reciprocal, broadcast across partitions via
  a DRAM bounce, ctxT normalized into SBUF.
  Projection chains (QK/V/O) are interleaved as PE fillers throughout; all
  PSUM->SBUF evacuations run on the otherwise-idle Pool (GpSimd) engine.
  out_partial = ctxT.T @ WoT, written bf16; host sums the two partials.
"""

import os
import sys

sys.path.insert(0, "/opt/trn_rl_repo")

import numpy as np
import ml_dtypes

bf16np = ml_dtypes.bfloat16

D, H, Dh, R = 1024, 16, 64, 16
S, B = 2048, 4
SCALING = 1.0 / R
N_CORES = 8

# schedule tuning knobs
SEG_ORDER = [
    (0, 0), (1, 0), (0, 1), (1, 1), (2, 0), (0, 2), (2, 1), (1, 2),
    (3, 0), (2, 2), (0, 3), (3, 1), (1, 3), (3, 2), (2, 3), (3, 3),
]
RATE_EARLY = 4.2   # filler matmuls per attention tile, early phase
RATE_LATE = 2.3
EARLY_TILES = 48
N_WARMUP = 19
USE_GPSIMD_DEN = False

_compiled = {}


def _build_nc():
    import concourse.bass as bass
    import concourse.tile as tile
    from concourse import mybir

    fp32 = mybir.dt.float32
    bf16 = mybir.dt.bfloat16

    nc = bass.Bass()

    xt_d = nc.dram_tensor("xt", [128, 4, 8, 512], bf16, kind="ExternalInput")
    wqt_d = nc.dram_tensor("wqt", [128, 8, 512], bf16, kind="ExternalInput")
    wkt_d = nc.dram_tensor("wkt", [128, 8, 512], bf16, kind="ExternalInput")
    wvt_d = nc.dram_tensor("wvt", [128, 8, 512], bf16, kind="ExternalInput")
    wot_d = nc.dram_tensor("wot", [128, 4, D], bf16, kind="ExternalInput")
    tri_d = nc.dram_tensor("tri", [128, 128], bf16, kind="ExternalInput")
    out_d = nc.dram_tensor("out", [16, 128, D], bf16, kind="ExternalOutput")

    with tile.TileContext(nc) as tc:
        with (
            tc.tile_pool(name="consts", bufs=1) as consts,
            tc.tile_pool(name="acts", bufs=1) as acts,
            tc.tile_pool(name="attn", bufs=4) as attn_pool,
            tc.tile_pool(name="accp", bufs=2) as accp,
            tc.tile_pool(name="small", bufs=2) as small,
            tc.tile_pool(name="ostage", bufs=3) as ostage,
            tc.tile_pool(name="ps_sc", bufs=2, space="PSUM") as ps_sc,
            tc.tile_pool(name="ps_ctx", bufs=2, space="PSUM") as ps_ctx,
            tc.tile_pool(name="ps_proj", bufs=2, space="PSUM") as ps_proj,
            tc.tile_pool(name="dram", bufs=2, space="DRAM") as dram,
        ):
            # ---- input DMAs in consumption-priority order ----
            tri = consts.tile([128, 128], bf16, tag="tri")
            nc.sync.dma_start(out=tri, in_=tri_d[:])
            wqt = consts.tile([128, 8, 512], bf16, tag="wqt")
            nc.sync.dma_start(out=wqt, in_=wqt_d[:])
            wkt = consts.tile([128, 8, 512], bf16, tag="wkt")
            nc.sync.dma_start(out=wkt, in_=wkt_d[:])
            xt = consts.tile([128, 4, 8, 512], bf16, tag="xt")
            nc.sync.dma_start(out=xt[:, 0], in_=xt_d[:, 0])
            wvt = consts.tile([128, 8, 512], bf16, tag="wvt")
            nc.sync.dma_start(out=wvt, in_=wvt_d[:])
            for tc_i in range(1, 4):
                nc.sync.dma_start(out=xt[:, tc_i], in_=xt_d[:, tc_i])
            wot = consts.tile([128, 4, D], bf16, tag="wot")
            nc.sync.dma_start(out=wot, in_=wot_d[:])

            ones = consts.tile([128, 8], bf16, tag="ones")
            nc.vector.memset(ones, 1.0)
            warm = consts.tile([128, 512], bf16, tag="warm")
            nc.vector.memset(warm, 0.5)

            # ---- PE warm-up: junk matmuls while DMAs land (p-state ramp,
            # and PE stays lightly busy until x arrives) ----
            warm_ps = ps_ctx.tile([128, 512], fp32, tag="ctx", name="warm_ps")
            for _ in range(N_WARMUP):
                nc.tensor.matmul(
                    warm_ps[0:64, :],
                    warm[:, 0:64],
                    warm,
                    start=True,
                    stop=True,
                    skip_group_check=True,
                )

            qt = acts.tile([128, 4, S], bf16, tag="qt")
            ktt = acts.tile([128, 4, S], bf16, tag="ktt")
            v = acts.tile([128, 16, 512], bf16, tag="v")
            ctxt = acts.tile([128, 4, S], bf16, tag="ctxt")

            # ---- projection chain generators (one yield per matmul; the
            # PSUM->SBUF evacuation runs on the Pool engine) ----
            def qk_chain(p, tb, which):
                w = wqt if which == "q" else wkt
                dst = qt if which == "q" else ktt
                ps = ps_proj.tile([128, 512], fp32, tag="proj", name="qk_ps")
                for k in range(8):
                    nc.tensor.matmul(
                        ps,
                        w[:, k, p * 128:(p + 1) * 128],
                        xt[:, tb, k, :],
                        start=(k == 0),
                        stop=(k == 7),
                    )
                    yield
                nc.vector.tensor_copy(dst[:, p, tb * 512:(tb + 1) * 512], ps)

            def v_chain(tt):
                tc_i, off = tt // 4, (tt % 4) * 128
                ps = ps_proj.tile([128, 512], fp32, tag="proj", name="v_ps")
                for k in range(8):
                    nc.tensor.matmul(
                        ps,
                        xt[:, tc_i, k, off:off + 128],
                        wvt[:, k, :],
                        start=(k == 0),
                        stop=(k == 7),
                    )
                    yield
                nc.vector.tensor_copy(v[:, tt, :], ps)

            def o_chain(qt_i, db):
                ps = ps_proj.tile([128, 512], fp32, tag="proj", name="o_ps")
                for gg in range(4):
                    nc.tensor.matmul(
                        ps,
                        ctxt[:, gg, qt_i * 128:(qt_i + 1) * 128],
                        wot[:, gg, db * 512:(db + 1) * 512],
                        start=(gg == 0),
                        stop=(gg == 3),
                    )
                    yield
                st = ostage.tile([128, 512], bf16, tag="ostage")
                nc.vector.tensor_copy(st, ps)
                nc.sync.dma_start(out=out_d[qt_i, :, db * 512:(db + 1) * 512], in_=st)

            # ---- attention segment generator: one (head-pair p, q-block qb),
            # one yield per 128-k tile ----
            def attn_segment(p, qb):
                acc = accp.tile([128, 2, 512], bf16, tag="acc")
                ctx_ps = ps_ctx.tile([128, 512], fp32, tag="ctx")
                kt_hi = 4 * (qb + 1)
                for kt in range(kt_hi):
                    j = kt - 4 * qb
                    c0 = 128 * j if j >= 0 else 0
                    sc = ps_sc.tile([128, 2, 512], fp32, tag="sc")
                    for s in range(2):
                        hp = slice(s * 64, (s + 1) * 64)
                        nc.tensor.matmul(
                            sc[:, s, c0:],
                            ktt[hp, p, kt * 128:(kt + 1) * 128],
                            qt[hp, p, qb * 512 + c0:(qb + 1) * 512],
                            start=True,
                            stop=True,
                            tile_position=(s * 64, 0),
                        )
                    at = attn_pool.tile([128, 2, 512], bf16, tag="at")
                    nc.scalar.activation(
                        out=at[:, :, c0:],
                        in_=sc[:, :, c0:],
                        func=mybir.ActivationFunctionType.Exp,
                    )
                    if j >= 0:
                        tri_b = bass.AP(
                            tensor=tri.tensor,
                            offset=tri.offset,
                            ap=[tri.ap[0], [0, 2], tri.ap[1]],
                        )
                        nc.vector.tensor_mul(
                            at[:, :, c0:c0 + 128], at[:, :, c0:c0 + 128], tri_b
                        )
                    first = kt == 0
                    last = kt == kt_hi - 1
                    for s in range(2):
                        for hh in range(2):
                            co = p * 128 + s * 64 + hh * 32
                            nc.tensor.matmul(
                                ctx_ps[s * 64 + hh * 32:s * 64 + (hh + 1) * 32, c0:],
                                v[:, kt, co:co + 32],
                                at[:, s, c0:],
                                start=first,
                                stop=last,
                                tile_position=(0, s * 64 + hh * 32),
                            )
                    # softmax denominator partial accumulation: diagonal
                    # tiles on the idle GpSimd engine, full tiles on VectorE
                    eng = nc.gpsimd if (j >= 0 and USE_GPSIMD_DEN) else nc.vector
                    if first:
                        eng.tensor_copy(acc, at)
                    else:
                        eng.tensor_add(
                            acc[:, :, c0:], acc[:, :, c0:], at[:, :, c0:]
                        )
                    yield
                # segment epilogue: den, 1/den, partition-broadcast, normalize
                aux = ps_proj.tile([128, 512], fp32, tag="proj", name="aux")
                for s in range(2):
                    nc.tensor.matmul(
                        aux[32 * s:32 * s + 1, :],
                        ones[:, 0:1],
                        acc[:, s, :],
                        start=True,
                        stop=True,
                        skip_group_check=True,
                        tile_position=(0, 32 * s),
                    )
                # 1/den = exp(-ln(den)) on ScalarE (both funcs share one ACT
                # table set; DVE reciprocal is too slow)
                ld = small.tile([33, 512], fp32, tag="ld")
                nc.scalar.activation(
                    out=ld, in_=aux[0:33, :], func=mybir.ActivationFunctionType.Ln
                )
                rec = small.tile([33, 512], fp32, tag="rec")
                nc.scalar.activation(
                    out=rec, in_=ld,
                    func=mybir.ActivationFunctionType.Exp, scale=-1.0,
                )
                dscr = dram.tile([2, 512], fp32, tag="dscr")
                nc.sync.dma_start(out=dscr[0:1, :], in_=rec[0:1, :])
                nc.sync.dma_start(out=dscr[1:2, :], in_=rec[32:33, :])
                bc_sb = small.tile([128, 512], fp32, tag="bcsb")
                nc.sync.dma_start(
                    out=bc_sb[0:64], in_=dscr[0:1, :].to_broadcast((64, 512))
                )
                nc.sync.dma_start(
                    out=bc_sb[64:128], in_=dscr[1:2, :].to_broadcast((64, 512))
                )
                nc.vector.tensor_mul(
                    ctxt[:, p, qb * 512:(qb + 1) * 512], ctx_ps, bc_sb
                )
                yield

            # ---- filler chain queue (consumed between attention tiles).
            # Each entry is (key, gate_qb, generator). op chains are gated on
            # the last segment of their q-block; a segment force-drains the
            # queue up to the chains it reads (writes must be EMITTED before
            # dependent reads or the tile framework sees stale data). ----
            qb_done = [False] * 4

            def chain_list():
                out = []
                out.append((("v", 4), None, v_chain(4)))
                out.append((("v", 5), None, v_chain(5)))
                out.append((("qk", 0, 1, "q"), None, qk_chain(0, 1, "q")))
                out.append((("qk", 0, 1, "k"), None, qk_chain(0, 1, "k")))
                out.append((("v", 6), None, v_chain(6)))
                out.append((("v", 7), None, v_chain(7)))
                out.append((("qk", 1, 1, "q"), None, qk_chain(1, 1, "q")))
                out.append((("qk", 1, 1, "k"), None, qk_chain(1, 1, "k")))
                for tb in range(2):
                    out.append((("qk", 2, tb, "q"), None, qk_chain(2, tb, "q")))
                    out.append((("qk", 2, tb, "k"), None, qk_chain(2, tb, "k")))
                for tb in range(2, 4):
                    for pp in (0, 1):
                        out.append((("qk", pp, tb, "q"), None, qk_chain(pp, tb, "q")))
                        out.append((("qk", pp, tb, "k"), None, qk_chain(pp, tb, "k")))
                for tb in range(2, 4):
                    out.append((("qk", 2, tb, "q"), None, qk_chain(2, tb, "q")))
                    out.append((("qk", 2, tb, "k"), None, qk_chain(2, tb, "k")))
                for tt in range(8, 12):
                    out.append((("v", tt), None, v_chain(tt)))
                for tb in range(4):
                    out.append((("qk", 3, tb, "q"), None, qk_chain(3, tb, "q")))
                    out.append((("qk", 3, tb, "k"), None, qk_chain(3, tb, "k")))
                for tt in range(12, 16):
                    out.append((("v", tt), None, v_chain(tt)))
                for qt_i in range(0, 4):
                    for db in range(2):
                        out.append((("op", qt_i, db), 0, o_chain(qt_i, db)))
                for qt_i in range(4, 8):
                    for db in range(2):
                        out.append((("op", qt_i, db), 1, o_chain(qt_i, db)))
                for qt_i in range(8, 12):
                    for db in range(2):
                        out.append((("op", qt_i, db), 2, o_chain(qt_i, db)))
                return out

            fq = chain_list()
            emitted_keys = set()
            fpos = [0]

            def emit_fillers(n, force_keys=None):
                """Advance the filler queue by n matmuls; if force_keys is
                given, keep going until all those chains are fully emitted."""
                emitted = 0
                while fpos[0] < len(fq):
                    key, gate, ch = fq[fpos[0]]
                    if force_keys is not None:
                        if force_keys <= emitted_keys:
                            break
                    elif emitted >= n:
                        break
                    if gate is not None and not qb_done[gate]:
                        if force_keys is not None and not (
                            force_keys <= emitted_keys
                        ):
                            # ops never appear in force_keys; stop at gate
                            break
                        break
                    try:
                        next(ch)
                        emitted += 1
                    except StopIteration:
                        emitted_keys.add(key)
                        fpos[0] += 1
                return emitted

            def seg_needs(p, qb):
                need = set()
                if qb >= 1:
                    need.add(("qk", p, qb, "q"))
                    for tb in range(1, qb + 1):
                        need.add(("qk", p, tb, "k"))
                    for tt in range(4, 4 * (qb + 1)):
                        need.add(("v", tt))
                if p >= 2:
                    need.add(("qk", p, 0, "q"))
                    need.add(("qk", p, 0, "k"))
                # only chains that exist in the queue
                return need & all_keys

            # ---- start block: first QKV chains (DMA-paced) ----
            for g in qk_chain(0, 0, "q"):
                pass
            for g in qk_chain(0, 0, "k"):
                pass
            for tt in range(4):
                for g in v_chain(tt):
                    pass
            for g in qk_chain(1, 0, "q"):
                pass
            for g in qk_chain(1, 0, "k"):
                pass
            all_keys = {key for key, _, _ in fq}

            # ---- main driver: two attention segments in flight + fillers ----
            def start_seg(p, qb):
                emit_fillers(0, force_keys=seg_needs(p, qb))
                return attn_segment(p, qb)

            slots = [start_seg(*SEG_ORDER[0]), start_seg(*SEG_ORDER[1])]
            slot_seg = [SEG_ORDER[0], SEG_ORDER[1]]
            nxt = 2
            tiles_done = 0
            credit = 0.0
            while any(s is not None for s in slots):
                for i in (0, 1):
                    seg = slots[i]
                    if seg is None:
                        continue
                    try:
                        next(seg)
                        tiles_done += 1
                        credit += RATE_EARLY if tiles_done < EARLY_TILES else RATE_LATE
                    except StopIteration:
                        p_f, qb_f = slot_seg[i]
                        if p_f == 3:
                            qb_done[qb_f] = True
                        if nxt < len(SEG_ORDER):
                            slot_seg[i] = SEG_ORDER[nxt]
                            slots[i] = start_seg(*SEG_ORDER[nxt])
                            nxt += 1
                        else:
                            slots[i] = None
                    if credit >= 1.0:
                        emit_fillers(int(credit))
                        credit -= int(credit)

            # drain remaining fillers, then the last o-projection block
            emit_fillers(10 ** 9)
            for qt_i in range(12, 16):
                for db in range(2):
                    for g in o_chain(qt_i, db):
                        pass

    _fix_matmul_waits(nc, mybir)
    return nc


_WAIT_LIMITS = {"InstISA": 0}


def _fix_matmul_waits(nc, mybir):
    """Walrus encodes at most one sync-wait command on compute-engine datapath
    instructions (MM/TT/ACT/...). Split excess waits into standalone
    InstEventSemaphore waits on the same engine immediately before the
    instruction — semantically identical (same engine stream, same point)."""
    import bass_rust

    counter = [0]

    def make_wait(engine, w):
        counter[0] += 1
        ev = mybir.InstEventSemaphore(name=f"W-split-{counter[0]}", ins=[], outs=[])
        ev.engine = engine
        ev.sync_info = bass_rust.SyncInfo(on_wait=[w], on_update=[])
        return ev

    for blk in nc.m.functions[0].blocks:
        insts = list(blk.instructions)
        out = []
        changed = False
        for ins in insts:
            si = ins.sync_info
            limit = _WAIT_LIMITS.get(type(ins).__name__, 1)
            if si is not None and len(si.on_wait) > limit:
                waits = list(si.on_wait)
                extra, keep = waits[:-limit], waits[-limit:]
                for w in extra:
                    out.append(make_wait(ins.engine, w))
                si.on_wait = keep
                ins.sync_info = si
                changed = True
            out.append(ins)
        if changed:
            blk.instructions = out


def _get_nc():
    if "nc" not in _compiled:
        _compiled["nc"] = _build_nc()
    return _compiled["nc"]


def _fold(w, a, b):
    return w.astype(np.float64) + SCALING * (
        b.astype(np.float64) @ a.astype(np.float64)
    )


def _prep_in_maps(inputs):
    x = np.asarray(inputs["x"], np.float32)
    wq_e = _fold(inputs["wq"], inputs["aq"], inputs["bq"])
    wk_e = _fold(inputs["wk"], inputs["ak"], inputs["bk"])
    wv_e = _fold(inputs["wv"], inputs["av"], inputs["bv"])
    wo_e = _fold(inputs["wo"], inputs["ao"], inputs["bo"])

    tri = np.triu(np.ones((128, 128), np.float32)).astype(bf16np)

    in_maps = []
    for c in range(N_CORES):
        b, g = c // 2, c % 2
        gs = slice(g * 512, (g + 1) * 512)
        # [ch128, tok-chunk 4, ch-chunk 8, tok 512]
        xt = (
            x[b].T.reshape(8, 128, 4, 512).transpose(1, 2, 0, 3).astype(bf16np)
        )
        wqt = (
            (wq_e[gs].T * 0.125).reshape(8, 128, 512).transpose(1, 0, 2).astype(bf16np)
        )
        wkt = wk_e[gs].T.reshape(8, 128, 512).transpose(1, 0, 2).astype(bf16np)
        wvt = wv_e[gs].T.reshape(8, 128, 512).transpose(1, 0, 2).astype(bf16np)
        wot = wo_e[:, gs].T.reshape(4, 128, D).transpose(1, 0, 2).astype(bf16np)
        in_maps.append(
            dict(
                xt=np.ascontiguousarray(xt),
                wqt=np.ascontiguousarray(wqt),
                wkt=np.ascontiguousarray(wkt),
                wvt=np.ascontiguousarray(wvt),
                wot=np.ascontiguousarray(wot),
                tri=tri,
            )
        )
    return in_maps


def run(inputs, trace=False, **kw):
    """Run on 8 cores; returns (full_output, BassKernelResults)."""
    from concourse.bass_utils import run_bass_kernel_spmd

    nc = _get_nc()
    in_maps = _prep_in_maps(inputs)
    res = run_bass_kernel_spmd(
        nc, in_maps, core_ids=list(range(N_CORES)), trace=trace, **kw
    )
    full = np.zeros((B, S, D), np.float32)
    for b in range(B):
        o0 = np.asarray(res.results[2 * b]["out"], np.float32).reshape(S, D)
        o1 = np.asarray(res.results[2 * b + 1]["out"], np.float32).reshape(S, D)
        full[b] = o0 + o1
    return full, res


def kernel(**inputs):
    full, _ = run(inputs, trace=False)
    return full
